# revision 28
# baseline (speedup 1.0000x reference)
"""Trainium2 Bass kernel: complex-valued transformer block (nn_EqModelComplex).

Sharding: 8 cores = (batch b in 0..3) x (query-half hh in 0..1); each core does
512 query tokens of one batch element, K/V over its full 1024-token sequence.
The K/V sequence is host-permuted per core to [own-half | other-half] so the
module is core-uniform and the query chunk is always K/V chunk 0 (its LN1
output is reused for Q — no separate query LayerNorm).

Feature-major layout, all-bf16 GEMMs. K/Q use a stacked per-head row order
[r_lo(0:32) | i_lo(32:64) | r_hi(64:96) | i_hi(96:128)] so Hermitian scores
take ONE matmul per (head, key-block) and RoPE's rotate-half is a contiguous
64-row shift. LayerNorm stats math runs on compact [128,4] tiles with a
magic-number rsqrt (no Sqrt activation tables). FFN gating uses
sigmoid(x) = 0.5 + 0.5*tanh(x/2) (tanh shares the exp table set).
Activation-table loads are minimized by batching all Exp then all Sin ops.
"""
import sys, os
sys.path.insert(0, '/opt/trn_rl_repo')
import math
import numpy as np
from contextlib import ExitStack

P = 128
D = 512
S = 1024
B = 4
H = 8
HD = 64
HID = 2048
TQ = 512
FT = D // P          # 4
NCORES = 8
EPS = 1e-6
SCALE = 1.0 / math.sqrt(HD)
MAGIC_F = float(np.frombuffer(np.uint32(0x5f3759df).tobytes(), dtype=np.float32)[0])

_CACHE = {}


def _emit_body(nc, tc, io, TRIVIAL_LN=False):
    from concourse import mybir

    dt = mybir.dt
    AF = mybir.ActivationFunctionType
    ALU = mybir.AluOpType
    f32 = dt.float32
    i32 = dt.int32
    bf16 = dt.bfloat16
    TT = nc.vector.tensor_tensor
    TS = nc.vector.tensor_scalar
    PTT = nc.gpsimd.tensor_tensor
    PTS = nc.gpsimd.tensor_scalar
    PTC = nc.gpsimd.tensor_copy
    ACT = nc.scalar.activation
    lnrow = [0]

    ctx = ExitStack()
    with ctx:
        # ---------------- long-lived pools ----------------
        const = ctx.enter_context(tc.tile_pool(name="const", bufs=1))
        p_bc = ctx.enter_context(tc.tile_pool(name="p_bc", bufs=1))
        ps = ctx.enter_context(tc.tile_pool(name="ps", bufs=4, space="PSUM"))
        ps_at = ctx.enter_context(tc.tile_pool(name="ps_at", bufs=2, space="PSUM"))

        ones16 = const.tile([P, 1], bf16)
        nc.vector.memset(ones16, 1.0)
        magic4 = const.tile([P, 4], f32)
        nc.vector.memset(magic4, MAGIC_F)

        lncols = {}
        if not TRIVIAL_LN:
            for key in ['ln1_gr', 'ln1_gi', 'ln1_br', 'ln1_bi',
                        'ln2_gr', 'ln2_gi', 'ln2_br', 'ln2_bi']:
                c = const.tile([P, FT], f32, name='c_' + key)
                nc.sync.dma_start(out=c, in_=io[key].rearrange("(t p) -> p t", p=P))
                lncols[key] = c

        # bias columns for v/o ([P, FT], feature-major) and permuted [P, H]
        # stacked-layout bias columns for q/k. Sin ops deferred into the
        # first sin batch (table-load discipline).
        bias_raw = {}
        for nm in ['q', 'k', 'v', 'o']:
            bm = const.tile([P, FT], f32, name='cb_bm_' + nm)
            bp = const.tile([P, FT], f32, name='cb_bp_' + nm)
            nc.sync.dma_start(out=bm, in_=io[nm + '_bm'].rearrange("(t p) -> p t", p=P))
            nc.sync.dma_start(out=bp, in_=io[nm + '_bp'].rearrange("(t p) -> p t", p=P))
            bias_raw[nm] = (bm, bp)
        bias_cols = {}     # nm -> (br, bi) [P, FT] f32
        bias_perm = {}     # nm -> [P, H] f32 (stacked row order), q/k only

        def emit_bias_sins():
            # runs inside a Sin table region; ONE [P, 4*FT] pair of Sin ops
            bpm = const.tile([P, 4 * FT], f32, name='cb_bpm')
            shm = const.tile([P, 4 * FT], f32, name='cb_shm')
            spm = const.tile([P, 4 * FT], f32, name='cb_spm')
            for i, nm in enumerate(['q', 'k', 'v', 'o']):
                nc.vector.tensor_copy(out=bpm[:, i * FT:(i + 1) * FT], in_=bias_raw[nm][1])
            ACT(out=shm, in_=bpm, func=AF.Sin, scale=0.5)
            ACT(out=spm, in_=bpm, func=AF.Sin)
            for i, nm in enumerate(['q', 'k', 'v', 'o']):
                bm = bias_raw[nm][0]
                sh = shm[:, i * FT:(i + 1) * FT]
                sp = spm[:, i * FT:(i + 1) * FT]
                br_ = const.tile([P, FT], f32, name='bias_r_' + nm)
                bi_ = const.tile([P, FT], f32, name='bias_i_' + nm)
                TT(out=sh, in0=sh, in1=sh, op=ALU.mult)
                TS(out=sh, in0=sh, scalar1=-2.0, scalar2=1.0, op0=ALU.mult, op1=ALU.add)
                TT(out=br_, in0=bm, in1=sh, op=ALU.mult)
                TT(out=bi_, in0=bm, in1=sp, op=ALU.mult)
                bias_cols[nm] = (br_, bi_)
            # permuted bias for q/k via scratch roundtrip:
            # stacked row p: half=p//64 (0 -> feats 0:32, 1 -> feats 32:64),
            # comp=(p%64)//32 (0 real, 1 imag), c=p%32; feat f = 64h+32half+c
            for qi, nm in enumerate(['q', 'k']):
                br_, bi_ = bias_cols[nm]
                rA, rB = 24 + 2 * qi, 25 + 2 * qi
                nc.sync.dma_start(
                    out=io['scratch'][rA, 0:D].rearrange("(t p) -> p t", p=P), in_=br_)
                nc.sync.dma_start(
                    out=io['scratch'][rB, 0:D].rearrange("(t p) -> p t", p=P), in_=bi_)
                bt = const.tile([P, H], f32, name='bias_perm_' + nm)
                vA = io['scratch'][rA, 0:D].rearrange("(h half c) -> half c h", h=H, half=2, c=32)
                vB = io['scratch'][rB, 0:D].rearrange("(h half c) -> half c h", h=H, half=2, c=32)
                nc.sync.dma_start(out=bt[0:32, :], in_=vA[0])
                nc.sync.dma_start(out=bt[32:64, :], in_=vB[0])
                nc.sync.dma_start(out=bt[64:96, :], in_=vA[1])
                nc.sync.dma_start(out=bt[96:128, :], in_=vB[1])
                bias_perm[nm] = bt

        # ---------- batched weight preprocessing ----------
        # Each entry: load lm/ph, mag=Exp(lm) [exp batch], s=Sin(ph/2),
        # snp=Sin(ph) [sin batch], then elementwise on DVE/Pool.
        def prep_batch(entries, tpool, extra_sins=None):
            # Mega-tile staging: per half-batch of <=4 entries, ONE Exp and
            # TWO Sin activation instructions over [P, 4*D] concatenations —
            # atomic ACT ops the scheduler cannot interleave across table
            # sets. Elementwise consumers slice the mega outputs.
            halves = [entries[i:i + 4] for i in range(0, len(entries), 4)]
            outs = []
            for hidx, hh in enumerate(halves):
                n = len(hh)
                lm_m = tpool.tile([P, n * D], f32, tag="wp_lm", bufs=1, name="lmm%d" % hidx, uniquify=True)
                for i, e in enumerate(hh):
                    e['loader_lm'](lm_m[:, i * D:(i + 1) * D])
                mag_m = tpool.tile([P, n * D], bf16, tag="wp_mag", bufs=2, name="magm%d" % hidx, uniquify=True)
                ACT(out=mag_m, in_=lm_m, func=AF.Exp)
                outs.append((hh, mag_m))
            souts = []
            for hidx, hh in enumerate(halves):
                n = len(hh)
                ph_m = tpool.tile([P, n * D], f32, tag="wp_ph", bufs=1, name="phm%d" % hidx, uniquify=True)
                for i, e in enumerate(hh):
                    e['loader_ph'](ph_m[:, i * D:(i + 1) * D])
                s_m = tpool.tile([P, n * D], bf16, tag="wp_s", bufs=1, name="sm%d" % hidx, uniquify=True)
                snp_m = tpool.tile([P, n * D], bf16, tag="wp_snp", bufs=1, name="snpm%d" % hidx, uniquify=True)
                ACT(out=s_m, in_=ph_m, func=AF.Sin, scale=0.5)
                ACT(out=snp_m, in_=ph_m, func=AF.Sin)
                souts.append((s_m, snp_m))
            if extra_sins is not None:
                extra_sins()
            for hidx, (hh, mag_m) in enumerate(outs):
                s_m, snp_m = souts[hidx]
                for i, e in enumerate(hh):
                    dsl = slice(i * D, (i + 1) * D)
                    mag = mag_m[:, dsl]
                    s_ = s_m[:, dsl]
                    snp = snp_m[:, dsl]
                    ssq = tpool.tile([P, D], bf16, tag="wp_ssq", bufs=1)
                    TT(out=ssq, in0=s_, in1=s_, op=ALU.mult)
                    TS(out=ssq, in0=ssq, scalar1=-2.0, scalar2=1.0, op0=ALU.mult, op1=ALU.add)
                    if e['kind'] == 'stacked':
                        W1, W2 = e['W1'], e['W2']
                        w1v = W1.rearrange("p (h half comp c) -> p h half comp c",
                                           h=H, half=2, comp=2, c=32)
                        w2v = W2.rearrange("p (h half comp c) -> p h half comp c",
                                           h=H, half=2, comp=2, c=32)
                        cv = ssq.rearrange("p (h half c) -> p h half c", h=H, half=2, c=32)
                        mv = mag.rearrange("p (h half c) -> p h half c", h=H, half=2, c=32)
                        sv = snp.rearrange("p (h half c) -> p h half c", h=H, half=2, c=32)
                        wr_v = w1v[:, :, :, 0]
                        wi_v = w1v[:, :, :, 1]
                        TT(out=wr_v, in0=cv, in1=mv, op=ALU.mult)          # mag*cos
                        TT(out=wi_v, in0=sv, in1=mv, op=ALU.mult)          # mag*sin
                        TS(out=w2v[:, :, :, 0], in0=wi_v, scalar1=-1.0, scalar2=None,
                           op0=ALU.mult)                                   # -wi
                        PTC(out=w2v[:, :, :, 1], in_=wr_v)                 # wr
                    else:
                        wr = e['wr']; wi = e['wi']; wineg = e['wineg']
                        TT(out=wr, in0=ssq, in1=mag, op=ALU.mult)
                        TT(out=wi, in0=snp, in1=mag, op=ALU.mult)
                        TS(out=wineg, in0=wi, scalar1=-1.0, scalar2=None, op0=ALU.mult)

        def _row_loader(lmT, phT, kt):
            def ld_lm(lm):
                nc.sync.dma_start(out=lm, in_=lmT[kt * P:(kt + 1) * P, :])
            def ld_ph(ph):
                nc.sync.dma_start(out=ph, in_=phT[kt * P:(kt + 1) * P, :])
            return ld_lm, ld_ph

        def plain_entries(nm, wpool, wtags=("pw_wr", "pw_wi", "pw_wng"), wbufs=4):
            lmT, phT = io[nm + '_lmT'], io[nm + '_phT']
            ents = []
            for kt in range(FT):
                wr = wpool.tile([P, D], bf16, tag=wtags[0], bufs=wbufs)
                wi = wpool.tile([P, D], bf16, tag=wtags[1], bufs=wbufs)
                wineg = wpool.tile([P, D], bf16, tag=wtags[2], bufs=wbufs)
                llm, lph = _row_loader(lmT, phT, kt)
                ents.append(dict(kind='plain', loader_lm=llm, loader_ph=lph,
                                 wr=wr, wi=wi, wineg=wineg))
            return ents

        def stacked_entries(nm, wpool, tagp):
            lmT, phT = io[nm + '_lmT'], io[nm + '_phT']
            ents = []
            for kt in range(FT):
                W1 = wpool.tile([P, 2 * D], bf16, tag=tagp + "1", bufs=4)
                W2 = wpool.tile([P, 2 * D], bf16, tag=tagp + "2", bufs=4)
                llm, lph = _row_loader(lmT, phT, kt)
                ents.append(dict(kind='stacked', loader_lm=llm, loader_ph=lph,
                                 W1=W1, W2=W2))
            return ents

        # ---------- compact feature-major layernorm ----------
        def ln_stats(xr16, xi16, tpool, row):
            NT = 512
            pss = [ps.tile([1, NT], f32, tag="ps_main", name="lnps%d" % q) for q in range(3)]
            for kt in range(FT):
                sq1 = tpool.tile([P, NT], bf16, tag="ln_sq1", bufs=2)
                sq2 = tpool.tile([P, NT], bf16, tag="ln_sq2", bufs=2)
                TT(out=sq1, in0=xr16[kt], in1=xr16[kt], op=ALU.mult)
                TT(out=sq2, in0=xi16[kt], in1=xi16[kt], op=ALU.mult)
                TT(out=sq1, in0=sq1, in1=sq2, op=ALU.add)
                first, last = kt == 0, kt == FT - 1
                nc.tensor.matmul(pss[0], ones16, xr16[kt], start=first, stop=last)
                nc.tensor.matmul(pss[1], ones16, xi16[kt], start=first, stop=last)
                nc.tensor.matmul(pss[2], ones16, sq1, start=first, stop=last)
            for q in range(3):
                st_ = tpool.tile([1, NT], f32, tag="ln_st", bufs=3)
                nc.vector.tensor_copy(out=st_, in_=pss[q])
                nc.sync.dma_start(out=io['scratch'][row + q, 0:NT][None, :], in_=st_)

        def ln_compact(row, tpool):
            NT = 512
            st = tpool.tile([P, 12], f32, tag="ln_cst", bufs=2)
            for q in range(3):
                nc.sync.dma_start(
                    out=st[:, 4 * q:4 * q + 4],
                    in_=io['scratch'][row + q, 0:NT].rearrange("(p f) -> p f", p=P))
            mr = tpool.tile([P, 4], f32, tag="ln_mr", bufs=2)
            mi = tpool.tile([P, 4], f32, tag="ln_mi", bufs=2)
            var = tpool.tile([P, 4], f32, tag="ln_var", bufs=2)
            t = tpool.tile([P, 4], f32, tag="ln_t", bufs=2)
            iv = tpool.tile([P, 4], f32, tag="ln_iv", bufs=2)
            TS(out=mr, in0=st[:, 0:4], scalar1=1.0 / D, scalar2=None, op0=ALU.mult)
            TS(out=mi, in0=st[:, 4:8], scalar1=1.0 / D, scalar2=None, op0=ALU.mult)
            TS(out=var, in0=st[:, 8:12], scalar1=1.0 / D, scalar2=EPS,
               op0=ALU.mult, op1=ALU.add)
            TT(out=t, in0=mr, in1=mr, op=ALU.mult)
            TT(out=var, in0=var, in1=t, op=ALU.subtract)
            TT(out=t, in0=mi, in1=mi, op=ALU.mult)
            TT(out=var, in0=var, in1=t, op=ALU.subtract)
            ivi = iv.bitcast(i32)
            TS(out=ivi, in0=var.bitcast(i32), scalar1=1, scalar2=None,
               op0=ALU.logical_shift_right)
            TS(out=ivi, in0=ivi, scalar1=-1, scalar2=None, op0=ALU.mult)
            TT(out=ivi, in0=ivi, in1=magic4.bitcast(i32), op=ALU.add)
            for _ in range(2):
                TT(out=t, in0=iv, in1=iv, op=ALU.mult)
                TT(out=t, in0=t, in1=var, op=ALU.mult)
                TS(out=t, in0=t, scalar1=-0.5, scalar2=1.5, op0=ALU.mult, op1=ALU.add)
                TT(out=iv, in0=iv, in1=t, op=ALU.mult)
            TT(out=mr, in0=mr, in1=iv, op=ALU.mult)   # mr*iv
            TT(out=mi, in0=mi, in1=iv, op=ALU.mult)
            c16 = tpool.tile([P, 12], bf16, tag="ln_c16", bufs=2)
            nc.vector.tensor_copy(out=c16[:, 0:4], in_=iv)
            nc.vector.tensor_copy(out=c16[:, 4:8], in_=mr)
            nc.vector.tensor_copy(out=c16[:, 8:12], in_=mi)
            for q in range(3):
                nc.sync.dma_start(
                    out=io['scratch16'][row + q, 0:NT].rearrange("(p f) -> p f", p=P),
                    in_=c16[:, 4 * q:4 * q + 4])
            iv_b = p_bc.tile([P, NT], bf16, tag="bc_iv")
            mrr_b = p_bc.tile([P, NT], bf16, tag="bc_mrr")
            mri_b = p_bc.tile([P, NT], bf16, tag="bc_mri")
            nc.sync.dma_start(out=iv_b, in_=io['scratch16'][row + 0, 0:NT][None, :].to_broadcast([P, NT]))
            nc.sync.dma_start(out=mrr_b, in_=io['scratch16'][row + 1, 0:NT][None, :].to_broadcast([P, NT]))
            nc.sync.dma_start(out=mri_b, in_=io['scratch16'][row + 2, 0:NT][None, :].to_broadcast([P, NT]))
            return iv_b, mrr_b, mri_b

        def ln_apply(xr16, xi16, iv_b, mrr_b, mri_b, ln, dst_r, dst_i, hpool, htag, tpool, hbufs):
            NT = 512
            for kt in range(FT):
                if hbufs == 0:
                    hr = hpool.tile([P, NT], bf16, name=htag + "r%d" % kt, uniquify=True)
                    hi = hpool.tile([P, NT], bf16, name=htag + "i%d" % kt, uniquify=True)
                else:
                    hr = hpool.tile([P, NT], bf16, tag=htag + "r", bufs=hbufs)
                    hi = hpool.tile([P, NT], bf16, tag=htag + "i", bufs=hbufs)
                if TRIVIAL_LN:
                    tr = tpool.tile([P, NT], bf16, tag="ln_tr", bufs=3)
                    ti = tpool.tile([P, NT], bf16, tag="ln_ti", bufs=3)
                    TT(out=tr, in0=xr16[kt], in1=iv_b, op=ALU.mult)
                    TT(out=ti, in0=xi16[kt], in1=iv_b, op=ALU.mult)
                    TT(out=hr, in0=tr, in1=mrr_b, op=ALU.subtract)
                    TT(out=hi, in0=ti, in1=mri_b, op=ALU.subtract)
                else:
                    nr = tpool.tile([P, NT], bf16, tag="ln_nr", bufs=2)
                    ni = tpool.tile([P, NT], bf16, tag="ln_ni", bufs=2)
                    TT(out=nr, in0=xr16[kt], in1=iv_b, op=ALU.mult)
                    TT(out=nr, in0=nr, in1=mrr_b, op=ALU.subtract)
                    TT(out=ni, in0=xi16[kt], in1=iv_b, op=ALU.mult)
                    TT(out=ni, in0=ni, in1=mri_b, op=ALU.subtract)
                    gr_c, gi_c = lncols[ln + '_gr'], lncols[ln + '_gi']
                    br_c, bi_c = lncols[ln + '_br'], lncols[ln + '_bi']
                    ta = tpool.tile([P, NT], bf16, tag="ln_ta", bufs=2)
                    tb = tpool.tile([P, NT], bf16, tag="ln_tb", bufs=2)
                    TS(out=ta, in0=nr, scalar1=gr_c[:, kt:kt + 1], scalar2=None, op0=ALU.mult)
                    TS(out=tb, in0=ni, scalar1=gi_c[:, kt:kt + 1], scalar2=None, op0=ALU.mult)
                    TT(out=ta, in0=ta, in1=tb, op=ALU.subtract)
                    TS(out=hr, in0=ta, scalar1=br_c[:, kt:kt + 1], scalar2=None, op0=ALU.add)
                    TS(out=ta, in0=nr, scalar1=gi_c[:, kt:kt + 1], scalar2=None, op0=ALU.mult)
                    TS(out=tb, in0=ni, scalar1=gr_c[:, kt:kt + 1], scalar2=None, op0=ALU.mult)
                    TT(out=ta, in0=ta, in1=tb, op=ALU.add)
                    TS(out=hi, in0=ta, scalar1=bi_c[:, kt:kt + 1], scalar2=None, op0=ALU.add)
                dst_r.append(hr)
                dst_i.append(hi)

        # rope on a stacked [128, NT] tile: rows [r_lo|i_lo|r_hi|i_hi],
        # rotate partner is p <-> p+64; signed sin table (host) carries signs.
        def rope_stacked(dst, src_ps, costab, sintab, bias_col, tpool):
            NT = dst.shape[-1] if hasattr(dst, 'shape') else 512
            pre = tpool.tile([P, 512], bf16, tag="rope_pre", bufs=2)
            tmp = tpool.tile([P, 512], bf16, tag="rope_tmp", bufs=2)
            ACT(out=pre, in_=src_ps, func=AF.Identity, bias=bias_col)
            TT(out=tmp[0:64, :], in0=pre[64:128, :], in1=sintab[64:128, :], op=ALU.mult)
            TT(out=tmp[64:128, :], in0=pre[0:64, :], in1=sintab[0:64, :], op=ALU.mult)
            TT(out=dst, in0=pre, in1=costab, op=ALU.mult)
            TT(out=dst, in0=dst, in1=tmp, op=ALU.add)

        def load_x16(name_r, name_i, csl, tpool, xpool, xtag, xbufs):
            xr16, xi16 = [], []
            for kt in range(FT):
                a16 = xpool.tile([P, 512], bf16, tag=xtag + "r", bufs=xbufs)
                b16 = xpool.tile([P, 512], bf16, tag=xtag + "i", bufs=xbufs)
                nc.sync.dma_start(out=a16, in_=io[name_r][kt * P:(kt + 1) * P, csl])
                nc.sync.dma_start(out=b16, in_=io[name_i][kt * P:(kt + 1) * P, csl])
                xr16.append(a16)
                xi16.append(b16)
            return xr16, xi16

        def cplx_mm(ps_r, ps_i, w3, kt, nkt, rhs_r, rhs_i, msl):
            wr, wi, wineg = w3
            first, last = kt == 0, kt == nkt - 1
            nc.tensor.matmul(ps_r, wr[:, msl], rhs_r, start=first, stop=False)
            nc.tensor.matmul(ps_r, wineg[:, msl], rhs_i, start=False, stop=last)
            nc.tensor.matmul(ps_i, wi[:, msl], rhs_r, start=first, stop=False)
            nc.tensor.matmul(ps_i, wr[:, msl], rhs_i, start=False, stop=last)

        # ================= persistent attention tensors =================
        # right-stack open order p_wo -> p_am -> p_qk; pops p_qk (post-
        # attention), p_am (mid-O), p_wo (after O GEMMs)
        es_wo = ExitStack()
        p_wo = es_wo.enter_context(tc.tile_pool(name="p_wo", bufs=1, side='right'))
        es_attn = ExitStack()
        p_am = es_attn.enter_context(tc.tile_pool(name="p_am", bufs=1, side='right'))
        es_qkv = ExitStack()
        p_qk = es_qkv.enter_context(tc.tile_pool(name="p_qk", bufs=1, side='right'))
        k_stk = [p_qk.tile([P, S], bf16, name='kstk%d' % h) for h in range(H)]
        q_stk = [p_qk.tile([P, TQ], bf16, name='qstk%d' % h) for h in range(H)]
        vaug = [p_qk.tile([P, H, 129], bf16, name='vaug%d' % t) for t in range(8)]

        # ================= Phase LN1 + K/V/Q =================
        es_hf = ExitStack()
        p_hf = es_hf.enter_context(tc.tile_pool(name="p_hf", bufs=1))
        with tc.tile_pool(name="p_kv", bufs=1) as p_kv, \
             tc.tile_pool(name="p_kc", bufs=1) as p_kc:
            ck = p_kc.tile([P, S], bf16, name='ck')
            sk_t = p_kc.tile([P, S], bf16, name='sk_t')
            cq = p_kc.tile([P, TQ], bf16, name='cq')
            sq_t = p_kc.tile([P, TQ], bf16, name='sq_t')
            nc.sync.dma_start(out=ck, in_=io['cosk'][:])
            nc.sync.dma_start(out=sk_t, in_=io['sink'][:])
            nc.sync.dma_start(out=cq, in_=io['cosq'][:])
            nc.sync.dma_start(out=sq_t, in_=io['sinq'][:])
            for va in vaug:
                nc.vector.memset(va[:, :, 64:65], 1.0)

            # LN1 over both chunks (chunk 0 = this core's query half)
            hf_r, hf_i = [], []
            for ch in range(2):
                csl = slice(ch * 512, (ch + 1) * 512)
                xr16, xi16 = load_x16('xf_r', 'xf_i', csl, p_kv, p_kv, "x16c%d" % ch, 4)
                row = lnrow[0]; lnrow[0] += 3
                ln_stats(xr16, xi16, p_kv, row)
                iv_b, mrr_b, mri_b = ln_compact(row, p_kv)
                ln_apply(xr16, xi16, iv_b, mrr_b, mri_b, 'ln1',
                         hf_r, hf_i, p_hf, "hf", p_kv, hbufs=8)

            # --- K + V weight prep (one exp batch + one sin batch) ---
            ek = stacked_entries('k', p_kv, "kW")
            ev = plain_entries('v', p_kv)
            prep_batch(ek + ev, p_kv, extra_sins=emit_bias_sins)
            Wk1 = [e['W1'] for e in ek]
            Wk2 = [e['W2'] for e in ek]
            wv = [(e['wr'], e['wi'], e['wineg']) for e in ev]

            # --- K projection + rope (stacked per head) ---
            for ch in range(2):
                csl = slice(ch * 512, (ch + 1) * 512)
                for h in range(H):
                    hsl = slice(h * P, (h + 1) * P)
                    pk = ps.tile([P, 512], f32, tag="ps_main")
                    for kt in range(FT):
                        nc.tensor.matmul(pk, Wk1[kt][:, hsl], hf_r[ch * 4 + kt],
                                         start=(kt == 0), stop=False)
                        nc.tensor.matmul(pk, Wk2[kt][:, hsl], hf_i[ch * 4 + kt],
                                         start=False, stop=(kt == FT - 1))
                    rope_stacked(k_stk[h][:, csl], pk, ck[:, csl], sk_t[:, csl],
                                 bias_perm['k'][:, h:h + 1], p_kv)

            # --- V projection (token-major) + vaug ---
            for tkt in range(8):
                ch, tk4 = tkt // 4, tkt % 4
                tsl = slice(tk4 * P, (tk4 + 1) * P)
                pr = ps.tile([P, D], f32, tag="ps_main")
                pi = ps.tile([P, D], f32, tag="ps_main")
                for kt in range(FT):
                    first, last = kt == 0, kt == FT - 1
                    hfr_t, hfi_t = hf_r[ch * 4 + kt], hf_i[ch * 4 + kt]
                    nc.tensor.matmul(pr, hfr_t[:, tsl], wv[kt][0], start=first, stop=False)
                    nc.tensor.matmul(pr, hfi_t[:, tsl], wv[kt][2], start=False, stop=last)
                    nc.tensor.matmul(pi, hfr_t[:, tsl], wv[kt][1], start=first, stop=False)
                    nc.tensor.matmul(pi, hfi_t[:, tsl], wv[kt][0], start=False, stop=last)
                nc.vector.tensor_copy(out=vaug[tkt][:, :, 0:64], in_=pr.rearrange("p (h d) -> p h d", h=H))
                nc.vector.tensor_copy(out=vaug[tkt][:, :, 65:129], in_=pi.rearrange("p (h d) -> p h d", h=H))

            # --- Q + O weight prep batched ---
            eq = stacked_entries('q', p_kv, "qW")
            eo = plain_entries('o', p_wo, wtags=("ow_r", "ow_i", "ow_n"), wbufs=4)
            prep_batch(eq + eo, p_kv)
            Wq1 = [e['W1'] for e in eq]
            Wq2 = [e['W2'] for e in eq]
            wo = [(e['wr'], e['wi'], e['wineg']) for e in eo]

            # --- Q projection + rope (queries = chunk 0 of hf) ---
            for h in range(H):
                hsl = slice(h * P, (h + 1) * P)
                pq = ps.tile([P, TQ], f32, tag="ps_main")
                for kt in range(FT):
                    nc.tensor.matmul(pq, Wq1[kt][:, hsl], hf_r[kt],
                                     start=(kt == 0), stop=False)
                    nc.tensor.matmul(pq, Wq2[kt][:, hsl], hf_i[kt],
                                     start=False, stop=(kt == FT - 1))
                rope_stacked(q_stk[h], pq, cq, sq_t,
                             bias_perm['q'][:, h:h + 1], p_kv)
        es_hf.close()   # free hf

        # ---- FFN weight-prep helpers (grp0 is prefetched pre-attention) ----
        def _ffn_loader(mT, osl):
            def ld(t):
                tv = t.rearrange("p (t q) -> p t q", q=P)
                for kt in range(FT):
                    nc.sync.dma_start(out=tv[:, kt, :], in_=mT[kt * P:(kt + 1) * P, osl])
            return ld

        def make_ffn_entries(grp, wpool, unique):
            ents = []
            for j in range(4):
                osl = slice((grp * 4 + j) * P, (grp * 4 + j + 1) * P)
                for wn in ['g', 'u']:
                    if unique:
                        wr = wpool.tile([P, D], bf16, name="f0%s_r%d" % (wn, j), uniquify=True)
                        wi = wpool.tile([P, D], bf16, name="f0%s_i%d" % (wn, j), uniquify=True)
                        wng = wpool.tile([P, D], bf16, name="f0%s_n%d" % (wn, j), uniquify=True)
                    else:
                        wr = wpool.tile([P, D], bf16, tag="fw%s_r%d" % (wn, j % 2), bufs=2)
                        wi = wpool.tile([P, D], bf16, tag="fw%s_i%d" % (wn, j % 2), bufs=2)
                        wng = wpool.tile([P, D], bf16, tag="fw%s_n%d" % (wn, j % 2), bufs=2)
                    ents.append(dict(kind='plain',
                                     loader_lm=_ffn_loader(io[wn + '_lmT'], osl),
                                     loader_ph=_ffn_loader(io[wn + '_phT'], osl),
                                     wr=wr, wi=wi, wineg=wng))
            dents = []
            for kt in range(grp * 4, grp * 4 + 4):
                if unique:
                    wr = wpool.tile([P, D], bf16, name="d0_r%d" % kt, uniquify=True)
                    wi = wpool.tile([P, D], bf16, name="d0_i%d" % kt, uniquify=True)
                    wng = wpool.tile([P, D], bf16, name="d0_n%d" % kt, uniquify=True)
                else:
                    wr = wpool.tile([P, D], bf16, tag="dw_r", bufs=4)
                    wi = wpool.tile([P, D], bf16, tag="dw_i", bufs=4)
                    wng = wpool.tile([P, D], bf16, tag="dw_n", bufs=4)
                dlm, dph = _row_loader(io['d_lmT'], io['d_phT'], kt)
                dents.append(dict(kind='plain', loader_lm=dlm, loader_ph=dph,
                                  wr=wr, wi=wi, wineg=wng))
            return ents, dents

        # ================= Phase attention =================
        attn_r = [p_am.tile([P, TQ], bf16, name='attnr%d' % ot) for ot in range(FT)]
        attn_i = [p_am.tile([P, TQ], bf16, name='attni%d' % ot) for ot in range(FT)]
        with tc.tile_pool(name="p_at", bufs=1) as p_at, \
             tc.tile_pool(name="p_pt", bufs=4) as p_pt:
            mask_t = []
            for tkt in range(8):
                m = p_at.tile([P, TQ], bf16, name='mask%d' % tkt)
                nc.sync.dma_start(out=m, in_=io['maskT'][tkt * P:(tkt + 1) * P, :])
                mask_t.append(m)
            bvr_c, bvi_c = bias_cols['v']
            for h in range(H):
                ot, prow = h // 2, 64 * (h % 2)
                po_r = ps_at.tile([65, TQ], f32, tag="at_r")
                po_i = ps_at.tile([64, TQ], f32, tag="at_i")
                for tkt in range(8):
                    tsl = slice(tkt * P, (tkt + 1) * P)
                    pst = ps.tile([P, TQ], f32, tag="ps_main")
                    nc.tensor.matmul(pst, k_stk[h][:, tsl], q_stk[h],
                                     start=True, stop=True)
                    p32 = p_pt.tile([P, TQ], bf16, tag="p_e%d" % (tkt % 2))
                    ACT(out=p32, in_=pst, func=AF.Exp, scale=SCALE)
                    TT(out=p32, in0=p32, in1=mask_t[tkt], op=ALU.mult)
                    nc.tensor.matmul(po_r, vaug[tkt][:, h, 0:65], p32,
                                     start=(tkt == 0), stop=(tkt == 7))
                    nc.tensor.matmul(po_i, vaug[tkt][:, h, 65:129], p32,
                                     start=(tkt == 0), stop=(tkt == 7))
                dn = p_pt.tile([1, TQ], f32, tag="dn_st", bufs=2)
                ACT(out=dn, in_=po_r[64:65, :], func=AF.Identity)
                nc.vector.reciprocal(out=dn, in_=dn)
                nc.sync.dma_start(out=io['scratch'][30, 0:TQ][None, :], in_=dn)
                rec = p_bc.tile([64, TQ], f32, tag="bc_rec", bufs=2)
                nc.sync.dma_start(out=rec,
                                  in_=io['scratch'][30, 0:TQ][None, :].to_broadcast([64, TQ]))
                ar_v = attn_r[ot][prow:prow + 64, :]
                ai_v = attn_i[ot][prow:prow + 64, :]
                TT(out=ar_v, in0=po_r[0:64, :], in1=rec, op=ALU.mult)
                TT(out=ai_v, in0=po_i[0:64, :], in1=rec, op=ALU.mult)
                TS(out=ar_v, in0=ar_v, scalar1=bvr_c[prow:prow + 64, ot:ot + 1],
                   scalar2=None, op0=ALU.add)
                TS(out=ai_v, in0=ai_v, scalar1=bvi_c[prow:prow + 64, ot:ot + 1],
                   scalar2=None, op0=ALU.add)
        es_qkv.close()   # free k_stk/q_stk/vaug

        # ================= Phase O-proj + residual + LN2 =================
        es_keep = ExitStack()
        p_keep = es_keep.enter_context(tc.tile_pool(name="p_keep", bufs=1))
        res_r = [p_keep.tile([P, TQ], f32, name='resr%d' % ot) for ot in range(FT)]
        res_i = [p_keep.tile([P, TQ], f32, name='resi%d' % ot) for ot in range(FT)]
        h2_r, h2_i = [], []
        with tc.tile_pool(name="p_o", bufs=1) as p_o:
            bo_r, bo_i = bias_cols['o']
            xr_t, xi_t = [], []
            for kt in range(FT):
                a = p_o.tile([P, TQ], f32, tag="xo_r", bufs=4)
                b_ = p_o.tile([P, TQ], f32, tag="xo_i", bufs=4)
                nc.sync.dma_start(out=a, in_=io['xq_r32'][kt * P:(kt + 1) * P, :])
                nc.sync.dma_start(out=b_, in_=io['xq_i32'][kt * P:(kt + 1) * P, :])
                xr_t.append(a)
                xi_t.append(b_)
            for ot in range(FT):
                msl = slice(ot * P, (ot + 1) * P)
                pr = ps.tile([P, TQ], f32, tag="ps_main")
                pi = ps.tile([P, TQ], f32, tag="ps_main")
                for kt in range(FT):
                    cplx_mm(pr, pi, wo[kt], kt, FT, attn_r[kt], attn_i[kt], msl)
                TS(out=res_r[ot], in0=pr, scalar1=bo_r[:, ot:ot + 1], scalar2=None, op0=ALU.add)
                TT(out=res_r[ot], in0=res_r[ot], in1=xr_t[ot], op=ALU.add)
                TS(out=res_i[ot], in0=pi, scalar1=bo_i[:, ot:ot + 1], scalar2=None, op0=ALU.add)
                TT(out=res_i[ot], in0=res_i[ot], in1=xi_t[ot], op=ALU.add)
            es_attn.close()   # free attn tiles
            es_wo.close()
            r16, i16 = [], []
            for ot in range(FT):
                a16 = p_o.tile([P, TQ], bf16, tag="r16", bufs=4)
                b16 = p_o.tile([P, TQ], bf16, tag="i16", bufs=4)
                PTC(out=a16, in_=res_r[ot])
                PTC(out=b16, in_=res_i[ot])
                r16.append(a16)
                i16.append(b16)
            row = lnrow[0]; lnrow[0] += 3
            ln_stats(r16, i16, p_o, row)
            iv_b, mrr_b, mri_b = ln_compact(row, p_o)
            ln_apply(r16, i16, iv_b, mrr_b, mri_b, 'ln2',
                     h2_r, h2_i, p_keep, "h2", p_o, hbufs=0)

        # ================= Phase FFN =================
        acc_r = [p_keep.tile([P, TQ], f32, name='accr%d' % ot) for ot in range(FT)]
        acc_i = [p_keep.tile([P, TQ], f32, name='acci%d' % ot) for ot in range(FT)]
        with tc.tile_pool(name="p_f", bufs=1) as p_f, \
             tc.tile_pool(name="p_fh", bufs=1) as p_fh:
            for grp in range(4):
                ents, dents = make_ffn_entries(grp, p_f, unique=False)
                # Prep in two waves of 2 j's (gate/up ring bufs=2); per-j
                # gating with Sqrt+Tanh (tanh shares the exp table set).
                hids = []
                for wave in range(2):
                    wents = ents[wave * 4:(wave + 1) * 4]
                    prep_batch(wents + (dents if wave == 1 else []), p_f)
                    for j in range(wave * 2, wave * 2 + 2):
                        gw = ents[2 * j]
                        uw = ents[2 * j + 1]
                        pgr = ps.tile([P, TQ], f32, tag="ps_main")
                        pgi = ps.tile([P, TQ], f32, tag="ps_main")
                        for kt in range(FT):
                            ksl = slice(kt * P, (kt + 1) * P)
                            cplx_mm(pgr, pgi, (gw['wr'][:, ksl], gw['wi'][:, ksl], gw['wineg'][:, ksl]),
                                    kt, FT, h2_r[kt], h2_i[kt], slice(0, P))
                        pur = ps.tile([P, TQ], f32, tag="ps_main")
                        pui = ps.tile([P, TQ], f32, tag="ps_main")
                        for kt in range(FT):
                            ksl = slice(kt * P, (kt + 1) * P)
                            cplx_mm(pur, pui, (uw['wr'][:, ksl], uw['wi'][:, ksl], uw['wineg'][:, ksl]),
                                    kt, FT, h2_r[kt], h2_i[kt], slice(0, P))
                        gr16 = p_f.tile([P, TQ], bf16, tag="st_gr", bufs=3)
                        gi16 = p_f.tile([P, TQ], bf16, tag="st_gi", bufs=3)
                        ur16 = p_f.tile([P, TQ], bf16, tag="st_ur", bufs=3)
                        ui16 = p_f.tile([P, TQ], bf16, tag="st_ui", bufs=3)
                        ACT(out=gr16, in_=pgr, func=AF.Identity)
                        ACT(out=gi16, in_=pgi, func=AF.Identity)
                        nc.vector.tensor_copy(out=ur16, in_=pur)
                        nc.vector.tensor_copy(out=ui16, in_=pui)
                        m2 = p_f.tile([P, TQ], bf16, tag="f_m2", bufs=2)
                        t2 = p_f.tile([P, TQ], bf16, tag="f_t2", bufs=2)
                        TT(out=m2, in0=gr16, in1=gr16, op=ALU.mult)
                        TT(out=t2, in0=gi16, in1=gi16, op=ALU.mult)
                        TT(out=m2, in0=m2, in1=t2, op=ALU.add)
                        ACT(out=m2, in_=m2, func=AF.Sqrt)
                        ACT(out=m2, in_=m2, func=AF.Tanh, scale=0.5)
                        TS(out=m2, in0=m2, scalar1=0.5, scalar2=0.5, op0=ALU.mult, op1=ALU.add)
                        gar = p_f.tile([P, TQ], bf16, tag="f_gar", bufs=2)
                        gai = p_f.tile([P, TQ], bf16, tag="f_gai", bufs=2)
                        TT(out=gar, in0=gr16, in1=m2, op=ALU.mult)
                        TT(out=gai, in0=gi16, in1=m2, op=ALU.mult)
                        hr = p_fh.tile([P, TQ], bf16, tag="hidr", bufs=4)
                        hi = p_fh.tile([P, TQ], bf16, tag="hidi", bufs=4)
                        ta = p_f.tile([P, TQ], bf16, tag="f_ta", bufs=2)
                        tb = p_f.tile([P, TQ], bf16, tag="f_tb", bufs=2)
                        TT(out=ta, in0=gar, in1=ur16, op=ALU.mult)
                        TT(out=tb, in0=gai, in1=ui16, op=ALU.mult)
                        TT(out=hr, in0=ta, in1=tb, op=ALU.subtract)
                        TT(out=ta, in0=gar, in1=ui16, op=ALU.mult)
                        TT(out=tb, in0=gai, in1=ur16, op=ALU.mult)
                        TT(out=hi, in0=ta, in1=tb, op=ALU.add)
                        hids.append((hr, hi))
                # --- down-projection for this group ---
                for ot in range(FT):
                    msl = slice(ot * P, (ot + 1) * P)
                    pr = ps.tile([P, TQ], f32, tag="ps_main")
                    pi = ps.tile([P, TQ], f32, tag="ps_main")
                    for j in range(4):
                        cplx_mm(pr, pi, (dents[j]['wr'], dents[j]['wi'], dents[j]['wineg']),
                                j, 4, hids[j][0], hids[j][1], msl)
                    if grp == 0:
                        nc.vector.tensor_copy(out=acc_r[ot], in_=pr)
                        nc.vector.tensor_copy(out=acc_i[ot], in_=pi)
                    else:
                        TT(out=acc_r[ot], in0=acc_r[ot], in1=pr, op=ALU.add)
                        TT(out=acc_i[ot], in0=acc_i[ot], in1=pi, op=ALU.add)
        for ot in range(FT):
            TT(out=acc_r[ot], in0=acc_r[ot], in1=res_r[ot], op=ALU.add)
            TT(out=acc_i[ot], in0=acc_i[ot], in1=res_i[ot], op=ALU.add)
            nc.sync.dma_start(out=io['out_r'][ot * P:(ot + 1) * P, :], in_=acc_r[ot])
            nc.sync.dma_start(out=io['out_i'][ot * P:(ot + 1) * P, :], in_=acc_i[ot])
        es_keep.close()


def _build_module(n_iters=1, trivial_ln=False):
    import concourse.tile as tile
    from concourse import bacc, mybir

    f32 = mybir.dt.float32
    bf16 = mybir.dt.bfloat16
    nc = bacc.Bacc(None, target_bir_lowering=False, debug=False)
    with tile.TileContext(nc) as tc:
        with tc.tile_pool(name="dram", bufs=1, space="DRAM") as dram:
            io = {}

            def din(name, shape, dtype=f32):
                io[name] = dram.tile(shape, dtype, kind='ExternalInput', name=name, uniquify=False)

            din('xf_r', [D, S], bf16); din('xf_i', [D, S], bf16)
            din('xq_r32', [D, TQ]); din('xq_i32', [D, TQ])
            for nm in ['q', 'k', 'v', 'o']:
                din(nm + '_lmT', [D, D]); din(nm + '_phT', [D, D])
                din(nm + '_bm', [D]); din(nm + '_bp', [D])
            din('g_lmT', [D, HID]); din('g_phT', [D, HID])
            din('u_lmT', [D, HID]); din('u_phT', [D, HID])
            din('d_lmT', [HID, D]); din('d_phT', [HID, D])
            for ln in ['ln1', 'ln2']:
                for q in ['gr', 'gi', 'br', 'bi']:
                    din(ln + '_' + q, [D])
            din('cosq', [P, TQ], bf16); din('sinq', [P, TQ], bf16)
            din('cosk', [P, S], bf16); din('sink', [P, S], bf16)
            din('maskT', [S, TQ], bf16)
            io['out_r'] = dram.tile([D, TQ], f32, kind='ExternalOutput', name='out_r', uniquify=False)
            io['out_i'] = dram.tile([D, TQ], f32, kind='ExternalOutput', name='out_i', uniquify=False)
            io['scratch'] = dram.tile([32, S], f32, name='scratch', uniquify=False)
            io['scratch16'] = dram.tile([16, S], bf16, name='scratch16', uniquify=False)

            if n_iters == 1:
                _emit_body(nc, tc, io, TRIVIAL_LN=trivial_ln)
            else:
                with tc.For_i(0, n_iters, 1):
                    _emit_body(nc, tc, io, TRIVIAL_LN=trivial_ln)
    nc.compile()
    return nc


def _host_inputs(x_real, x_imag, full, core):
    import ml_dtypes
    b, hh = core // 2, core % 2
    qs = hh * TQ
    m = {}
    # K/V sequence order: [own half | other half]
    perm = np.concatenate([np.arange(qs, qs + TQ), np.arange((1 - hh) * TQ, (1 - hh) * TQ + TQ)])
    m['xf_r'] = np.ascontiguousarray(x_real[b][perm].T).astype(ml_dtypes.bfloat16)
    m['xf_i'] = np.ascontiguousarray(x_imag[b][perm].T).astype(ml_dtypes.bfloat16)
    m['xq_r32'] = np.ascontiguousarray(x_real[b, qs:qs + TQ, :].T)
    m['xq_i32'] = np.ascontiguousarray(x_imag[b, qs:qs + TQ, :].T)
    for nm in ['q', 'k', 'v', 'o']:
        m[nm + '_lmT'] = np.ascontiguousarray(full[nm + '_lm'].T)
        m[nm + '_phT'] = np.ascontiguousarray(full[nm + '_ph'].T)
        m[nm + '_bm'] = full[nm + '_bm']
        m[nm + '_bp'] = full[nm + '_bp']
    m['g_lmT'] = np.ascontiguousarray(full['gate_lm'].T)
    m['g_phT'] = np.ascontiguousarray(full['gate_ph'].T)
    m['u_lmT'] = np.ascontiguousarray(full['up_lm'].T)
    m['u_phT'] = np.ascontiguousarray(full['up_ph'].T)
    m['d_lmT'] = np.ascontiguousarray(full['down_lm'].T)
    m['d_phT'] = np.ascontiguousarray(full['down_ph'].T)
    for ln in ['ln1', 'ln2']:
        for q in ['gr', 'gi', 'br', 'bi']:
            m[ln + '_' + q] = full[ln + '_' + q]
    # rope tables for stacked row order: row p -> freq index c = p % 32;
    # signed sin: rows 0:64 -> +sin, rows 64:128 -> -sin
    invf = 1.0 / (10000.0 ** (np.arange(0, HD, 2, dtype=np.float64) / HD))
    cidx = np.arange(P) % 32
    sign = np.where(np.arange(P) < 64, 1.0, -1.0)
    pos_q = np.arange(qs, qs + TQ, dtype=np.float64)
    pos_k = perm.astype(np.float64)
    angq = pos_q[None, :] * invf[cidx][:, None]
    angk = pos_k[None, :] * invf[cidx][:, None]
    m['cosq'] = np.cos(angq).astype(ml_dtypes.bfloat16)
    m['sinq'] = (np.sin(angq) * sign[:, None]).astype(ml_dtypes.bfloat16)
    m['cosk'] = np.cos(angk).astype(ml_dtypes.bfloat16)
    m['sink'] = (np.sin(angk) * sign[:, None]).astype(ml_dtypes.bfloat16)
    tk = perm[:, None]
    tq = (qs + np.arange(TQ))[None, :]
    m['maskT'] = (tq >= tk).astype(ml_dtypes.bfloat16)
    return m


def kernel(**inputs):
    from concourse.bass_utils import run_bass_kernel_spmd

    full = {k: np.asarray(v, dtype=np.float32) for k, v in inputs.items()}
    x_real, x_imag = full['x_real'], full['x_imag']

    trivial = np.all(full['ln1_gr'] == 1) and np.all(full['ln2_gr'] == 1) and \
        all(np.all(full[k] == 0) for k in ['ln1_gi', 'ln1_br', 'ln1_bi',
                                           'ln2_gi', 'ln2_br', 'ln2_bi'])
    key = ('mod', trivial)
    if key not in _CACHE:
        _CACHE[key] = _build_module(1, trivial_ln=trivial)
    nc = _CACHE[key]

    in_maps = [_host_inputs(x_real, x_imag, full, c) for c in range(NCORES)]
    res = run_bass_kernel_spmd(nc, in_maps, core_ids=list(range(NCORES)), trace=False)

    out = np.empty((2, B, S, D), dtype=np.float32)
    for c in range(NCORES):
        b, hh = c // 2, c % 2
        qs = hh * TQ
        out[0, b, qs:qs + TQ, :] = res.results[c]['out_r'].T
        out[1, b, qs:qs + TQ, :] = res.results[c]['out_i'].T
    return out
